# revision 7
# baseline (speedup 1.0000x reference)
"""MoE-with-DeepGEMM kernel for 8 Trainium2 NeuronCores.

Problem: M=4096 tokens, D=2048 in-dim, H=2048 out-dim, E=8 experts.
    gate = softmax(x @ gate_w.T + gate_b)            # [M, E], fp32
    y    = (q8(x) @ q8(expert_w[e]).T) -> bf16       # [E, M, H]
    out  = sum_e gate[:, e, None] * y[e].astype(f32) # [M, H]

Strategy: data-parallel over tokens (M). Each of the 8 cores gets
M/8 = 512 tokens, all 8 experts' weights, and computes its output slice
independently — no collectives; the host concatenates the slices.

Per-core device work:
  - gating matmul in float32r (x^T stationary, gate_w^T moving),
    softmax on DVE/ACT in [m-partition, e-free] layout,
  - main GEMM in fp8 (e4m3) with perf_mode=DoubleRow (256-deep
    contraction per matmul), accumulating in PSUM f32,
  - PSUM -> bf16 (matches the reference's bf16 cast of y) on ACT,
  - acc += gate * y_bf16 fused on DVE (scalar_tensor_tensor).

Host-side prep (not device work): fp8 quantize (identical RNE cast the
reference performs), transposes so the contraction dim lands on SBUF
partitions, and the final concat of per-core outputs.
"""

import numpy as np
import ml_dtypes

import concourse.bacc as bacc
import concourse.bass as bass
import concourse.mybir as mybir
import concourse.tile as tile
from concourse.bass_utils import run_bass_kernel_spmd

M, D, H, E = 4096, 2048, 2048, 8
NCORES = 8
MS = M // NCORES          # tokens per core (512)
MC = MS // 128            # m-chunks of 128 partitions (4)
DS = D // 128             # d-subtiles of 128 (16)
KP = DS // 2              # DoubleRow d-pairs of 256 (8)
NH = 512                  # h columns per matmul (one PSUM bank of f32)
HC = H // NH              # h-chunks (4)

_NC = None


def _build_program() -> bass.Bass:
    dt = mybir.dt
    nc = bacc.Bacc(None, target_bir_lowering=False)

    xq = nc.dram_tensor("xq", [D, MS], dt.float8e4, kind="ExternalInput")
    xf = nc.dram_tensor("xf", [D, MS], dt.float32r, kind="ExternalInput")
    wq = nc.dram_tensor("wq", [E * D, H], dt.float8e4, kind="ExternalInput")
    gwt = nc.dram_tensor("gwt", [D, E], dt.float32r, kind="ExternalInput")
    gb = nc.dram_tensor("gb", [128, E], dt.float32, kind="ExternalInput")
    out = nc.dram_tensor("out", [MS, H], dt.float32, kind="ExternalOutput")

    with tile.TileContext(nc) as tc, \
            tc.tile_pool(name="const", bufs=1) as constp, \
            tc.tile_pool(name="wpool", bufs=2) as wpool, \
            tc.tile_pool(name="ypool", bufs=6) as ypool, \
            tc.tile_pool(name="small", bufs=8) as small, \
            tc.tile_pool(name="ps", bufs=8, space="PSUM") as psp:

        # Persistent SBUF tensors. Contraction index d = s*128 + p.
        xq_sb = constp.tile([128, DS, MS], dt.float8e4, tag="xq")
        xf_sb = constp.tile([128, DS, MS], dt.float32r, tag="xf")
        gwt_sb = constp.tile([128, DS, E], dt.float32r, tag="gwt")
        gb_sb = constp.tile([128, E], dt.float32, tag="gb")
        gate_sb = constp.tile([128, MC * E], dt.float32, tag="gate")
        acc_sb = constp.tile([128, MC * H], dt.float32, tag="acc")

        nc.sync.dma_start(gwt_sb[:], gwt[:, :].rearrange("(s p) e -> p s e", p=128))
        nc.sync.dma_start(gb_sb[:], gb[:, :])
        nc.sync.dma_start(xf_sb[:], xf[:, :].rearrange("(s p) m -> p s m", p=128))
        nc.sync.dma_start(xq_sb[:], xq[:, :].rearrange("(s p) m -> p s m", p=128))

        # ---- Gating: logits -> softmax -> gate_sb[:, mc*E + e] ----
        for mc in range(MC):
            ps_g = psp.tile([128, E], dt.float32, tag="ps")
            msl = slice(mc * 128, (mc + 1) * 128)
            for s in range(DS):
                nc.tensor.matmul(
                    ps_g[:],
                    lhsT=xf_sb[:, s:s + 1, msl],
                    rhs=gwt_sb[:, s:s + 1, :],
                    start=(s == 0),
                    stop=(s == DS - 1),
                )
            logits = small.tile([128, E], dt.float32, tag="sm")
            nc.vector.tensor_add(logits[:], ps_g[:], gb_sb[:])
            mx = small.tile([128, 1], dt.float32, tag="sm1")
            nc.vector.tensor_reduce(
                mx[:], logits[:], mybir.AxisListType.X, mybir.AluOpType.max
            )
            nmx = small.tile([128, 1], dt.float32, tag="sm1")
            nc.vector.tensor_scalar_mul(nmx[:], mx[:], -1.0)
            ex = small.tile([128, E], dt.float32, tag="sm")
            ssum = small.tile([128, 1], dt.float32, tag="sm1")
            nc.scalar.activation(
                ex[:], logits[:], mybir.ActivationFunctionType.Exp,
                bias=nmx[:], scale=1.0, accum_out=ssum[:],
            )
            rcp = small.tile([128, 1], dt.float32, tag="sm1")
            nc.vector.reciprocal(rcp[:], ssum[:])
            nc.vector.tensor_scalar_mul(gate_sb[:, mc * E:(mc + 1) * E], ex[:], rcp[:])

        # ---- Main GEMM + weighted combine ----
        for e in range(E):
            w_sb = wpool.tile([128, DS, H], dt.float8e4, tag="w")
            nc.sync.dma_start(
                w_sb[:],
                wq[e * D:(e + 1) * D, :].rearrange("(s p) h -> p s h", p=128),
            )
            for mc in range(MC):
                msl = slice(mc * 128, (mc + 1) * 128)
                pss = [
                    psp.tile([128, NH], dt.float32, tag="ps", name=f"ps_{e}_{mc}_{i}")
                    for i in range(HC)
                ]
                for k in range(KP):
                    lhsT = xq_sb[:, 2 * k:2 * k + 2, msl]
                    for hc in range(HC):
                        nc.tensor.matmul(
                            pss[hc][:],
                            lhsT=lhsT,
                            rhs=w_sb[:, 2 * k:2 * k + 2, hc * NH:(hc + 1) * NH],
                            start=(k == 0),
                            stop=(k == KP - 1),
                            perf_mode=mybir.MatmulPerfMode.DoubleRow,
                        )
                g_ap = gate_sb[:, mc * E + e:mc * E + e + 1]
                for hc in range(HC):
                    y = ypool.tile([128, NH], dt.bfloat16, tag="y")
                    nc.scalar.copy(y[:], pss[hc][:])
                    a_ap = acc_sb[:, mc * H + hc * NH:mc * H + (hc + 1) * NH]
                    if e == 0:
                        nc.vector.tensor_scalar_mul(a_ap, y[:], g_ap)
                    else:
                        nc.vector.scalar_tensor_tensor(
                            a_ap, y[:], g_ap, a_ap,
                            op0=mybir.AluOpType.mult, op1=mybir.AluOpType.add,
                        )
                if e == E - 1:
                    nc.sync.dma_start(
                        out[mc * 128:(mc + 1) * 128, :],
                        acc_sb[:, mc * H:(mc + 1) * H],
                    )

    nc.compile()
    return nc


def _get_nc() -> bass.Bass:
    global _NC
    if _NC is None:
        _NC = _build_program()
    return _NC


def _prep_in_maps(x, gate_w, gate_b, expert_w):
    f8fn = ml_dtypes.float8_e4m3fn
    f8trn = ml_dtypes.float8_e4m3  # same bits as e4m3fn for |v| <= 240

    x = np.asarray(x, dtype=np.float32)
    gate_w = np.asarray(gate_w, dtype=np.float32)
    gate_b = np.asarray(gate_b, dtype=np.float32)
    expert_w = np.asarray(expert_w, dtype=np.float32)

    # x^T: [D, M]; quantized and full-precision copies.
    xT = np.ascontiguousarray(x.T)                       # [D, M] f32
    xqT = xT.astype(f8fn).view(f8trn)                    # [D, M] fp8
    # expert_w [E, H, D] -> w^T per expert [E, D, H], quantized, stacked.
    wqT = np.ascontiguousarray(
        expert_w.transpose(0, 2, 1)
    ).astype(f8fn).view(f8trn).reshape(E * D, H)
    gwt = np.ascontiguousarray(gate_w.T)                 # [D, E] f32
    gbb = np.ascontiguousarray(np.broadcast_to(gate_b[None, :], (128, E)))

    in_maps = []
    for c in range(NCORES):
        csl = slice(c * MS, (c + 1) * MS)
        in_maps.append({
            "xq": np.ascontiguousarray(xqT[:, csl]),
            "xf": np.ascontiguousarray(xT[:, csl]),
            "wq": wqT,
            "gwt": gwt,
            "gb": gbb,
        })
    return in_maps


def kernel(x, gate_w, gate_b, expert_w, _trace=False, _trace_kwargs=None):
    nc = _get_nc()
    in_maps = _prep_in_maps(x, gate_w, gate_b, expert_w)
    kw = {}
    if _trace:
        kw["trace"] = True
        kw.update(_trace_kwargs or {})
    res = run_bass_kernel_spmd(nc, in_maps, core_ids=list(range(NCORES)), **kw)
    outp = np.concatenate(
        [np.asarray(res.results[c]["out"]) for c in range(NCORES)], axis=0
    )
    if _trace:
        return outp, res
    return outp
